# revision 2
# baseline (speedup 1.0000x reference)
"""FP8 block-quantized linear (nn_FP8Linear) on 8 trn2 NeuronCores.

y = dequant(fp8_blockquant(x)) @ dequant(fp8(weight) * block_scales).T + bias

Sharding: column-parallel (tensor-parallel along out_features N).
N = 11008 = 86 blocks of 128; padded to 88 blocks so each of the 8 cores
handles 11 N-blocks (1408 rows). x is replicated; no collectives.

Device kernel (per core):
  - weight: load fp32 -> cast fp8e4 (RNE, exact match with OCP e4m3fn for
    |w|<=240) -> multiply per-(128x128)-block scales -> bf16 -> bounce via
    DRAM -> DMA-xbar-transpose into K-major resident tiles (128K, 32kb, Nchunk).
  - x per 128-row M-tile: load fp32 -> per-(row,128-block) absmax ->
    scales 224/max (halved vs the reference's 448/max so values fit TRN
    fp8e4's +-240 range; the fp8 grid is relative so the quantization is
    identical) -> quantize to fp8e4 -> dequantize*max/224 -> bf16 ->
    DMA-xbar-transpose to (128K, 32kb, 128M) -> 96 accumulating matmuls
    (3 PSUM n-chunks x 32 k-blocks) -> +bias -> store fp32.
"""

import numpy as np

# ---- problem constants (hardcoded per contract) ----
M2D = 8192        # 4*2048 flattened tokens
K = 4096
N_FULL = 11008
N_CORES = 8
NBLK = 11         # N-blocks of 128 per core (after padding 86 -> 88)
N_SH = NBLK * 128  # 1408
KB = K // 128     # 32
FP8_HW_MAX = 224.0  # reference uses 448 (e4m3fn); TRN fp8e4 tops at 240,
                    # so quantize at half scale and dequantize by 2x scale.

_CACHE = {}


def _chunks(n_sh):
    out = []
    off = 0
    while off < n_sh:
        w = min(512, n_sh - off)
        out.append((off, w))
        off += w
    return out


def _build_nc(m2d=M2D, nblk=NBLK):
    import concourse.mybir as mybir
    import concourse.tile as tile
    from concourse import bacc

    f32 = mybir.dt.float32
    bf16 = mybir.dt.bfloat16
    f8 = mybir.dt.float8e4

    n_sh = nblk * 128
    chunks = _chunks(n_sh)
    n_mt = m2d // 128

    nc = bacc.Bacc("TRN2", target_bir_lowering=False, debug=False,
                   num_devices=N_CORES)

    x_d = nc.dram_tensor("x", (m2d, K), f32, kind="ExternalInput")
    w_d = nc.dram_tensor("w", (n_sh, K), f32, kind="ExternalInput")
    wsr_d = nc.dram_tensor("wsr", (128, nblk * KB), f32, kind="ExternalInput")
    biasr_d = nc.dram_tensor("biasr", (128, n_sh), f32, kind="ExternalInput")
    y_d = nc.dram_tensor("y", (m2d, n_sh), f32, kind="ExternalOutput")
    wdq_d = nc.dram_tensor("wdq_scratch", (n_sh, K), bf16)

    x_ap = x_d.ap()
    w_ap = w_d.ap()
    y_ap = y_d.ap()
    wdq_ap = wdq_d.ap()

    with tile.TileContext(nc) as tc:
        with (
            tc.tile_pool(name="const", bufs=1) as constp,
            tc.tile_pool(name="wT", bufs=1) as wTp,
            tc.tile_pool(name="ld", bufs=2) as ldp,
            tc.tile_pool(name="q8", bufs=2) as q8p,
            tc.tile_pool(name="deq", bufs=2) as deqp,
            tc.tile_pool(name="xT", bufs=2) as xTp,
            tc.tile_pool(name="sc", bufs=2) as scp,
            tc.tile_pool(name="yo", bufs=2) as yop,
            tc.tile_pool(name="ps", bufs=6, space="PSUM") as psp,
        ):
            # constants (pre-replicated across partitions on host)
            ws_sb = constp.tile([128, nblk * KB], f32)
            nc.sync.dma_start(out=ws_sb[:], in_=wsr_d.ap())
            bias_sb = constp.tile([128, n_sh], f32)
            nc.sync.dma_start(out=bias_sb[:], in_=biasr_d.ap())

            # ---- weight pipeline: quantize+dequant, bounce to DRAM bf16 ----
            for nb in range(nblk):
                wld = ldp.tile([128, K], f32, tag="ld")
                nc.sync.dma_start(out=wld[:], in_=w_ap[nb * 128:(nb + 1) * 128, :])
                wq = q8p.tile([128, K], f8, tag="q8")
                nc.vector.tensor_copy(out=wq[:], in_=wld[:])
                wdq = deqp.tile([128, K], bf16, tag="deq")
                nc.vector.tensor_tensor(
                    out=wdq.rearrange("p (a b) -> p a b", b=128),
                    in0=wq.rearrange("p (a b) -> p a b", b=128),
                    in1=ws_sb[:, nb * KB:(nb + 1) * KB][:, :, None].broadcast_to(
                        [128, KB, 128]),
                    op=mybir.AluOpType.mult,
                )
                nc.sync.dma_start(out=wdq_ap[nb * 128:(nb + 1) * 128, :], in_=wdq[:])

            # transpose K-major from DRAM: (Wn, 4096) -> (128, 32, Wn)
            wT = []
            for j, (off, wd) in enumerate(chunks):
                t = wTp.tile([128, KB, wd], bf16, tag=f"wT{j}")
                nc.scalar.dma_start(out=t[:], in_=wdq_ap[off:off + wd, :],
                                    transpose=True)
                wT.append(t)

            # ---- main M loop ----
            for mt in range(n_mt):
                xld = ldp.tile([128, K], f32, tag="ld")
                nc.sync.dma_start(out=xld[:], in_=x_ap[mt * 128:(mt + 1) * 128, :])

                mx = scp.tile([128, KB], f32, tag="mx")
                nc.vector.tensor_reduce(
                    out=mx[:],
                    in_=xld.rearrange("p (a b) -> p a b", b=128),
                    axis=mybir.AxisListType.X,
                    op=mybir.AluOpType.max,
                    apply_absolute_value=True,
                )
                # guard all-zero blocks (reference would NaN; inputs are randn)
                nc.vector.tensor_scalar(out=mx[:], in0=mx[:], scalar1=1e-30,
                                        scalar2=None, op0=mybir.AluOpType.max)
                r2 = scp.tile([128, KB], f32, tag="r2")
                nc.vector.reciprocal(r2[:], mx[:])
                nc.vector.tensor_scalar(out=r2[:], in0=r2[:], scalar1=FP8_HW_MAX,
                                        scalar2=None, op0=mybir.AluOpType.mult)
                s2 = scp.tile([128, KB], f32, tag="s2")
                nc.vector.tensor_scalar(out=s2[:], in0=mx[:],
                                        scalar1=1.0 / FP8_HW_MAX,
                                        scalar2=None, op0=mybir.AluOpType.mult)

                xq = q8p.tile([128, K], f8, tag="q8")
                nc.vector.tensor_tensor(
                    out=xq.rearrange("p (a b) -> p a b", b=128),
                    in0=xld.rearrange("p (a b) -> p a b", b=128),
                    in1=r2[:, :, None].broadcast_to([128, KB, 128]),
                    op=mybir.AluOpType.mult,
                )
                xdq = deqp.tile([128, K], bf16, tag="deq")
                nc.vector.tensor_tensor(
                    out=xdq.rearrange("p (a b) -> p a b", b=128),
                    in0=xq.rearrange("p (a b) -> p a b", b=128),
                    in1=s2[:, :, None].broadcast_to([128, KB, 128]),
                    op=mybir.AluOpType.mult,
                )
                xT = xTp.tile([128, KB, 128], bf16, tag="xT")
                nc.scalar.dma_start(out=xT[:], in_=xdq[:], transpose=True)

                yo = yop.tile([128, n_sh], f32, tag="yo")
                pss = [psp.tile([128, 512], f32, tag="ps", name=f"ps_{mt}_{j}")
                       for j in range(len(chunks))]
                for kb in range(KB):
                    for j, (off, wd) in enumerate(chunks):
                        nc.tensor.matmul(
                            pss[j][:, :wd],
                            xT[:, kb, :],
                            wT[j][:, kb, :],
                            start=(kb == 0),
                            stop=(kb == KB - 1),
                        )
                for j, (off, wd) in enumerate(chunks):
                    nc.vector.tensor_add(yo[:, off:off + wd], pss[j][:, :wd],
                                         bias_sb[:, off:off + wd])
                nc.sync.dma_start(out=y_ap[mt * 128:(mt + 1) * 128, :], in_=yo[:])

    nc.compile()
    return nc


def _get_nc(m2d=M2D, nblk=NBLK):
    key = (m2d, nblk)
    if key not in _CACHE:
        _CACHE[key] = _build_nc(m2d, nblk)
    return _CACHE[key]


def _make_in_maps(x, weight, weight_scale, bias):
    x2d = np.ascontiguousarray(
        np.asarray(x, dtype=np.float32).reshape(M2D, K))
    w = np.asarray(weight, dtype=np.float32)
    ws = np.asarray(weight_scale, dtype=np.float32)
    b = np.asarray(bias, dtype=np.float32)

    n_pad = N_CORES * N_SH
    w_pad = np.zeros((n_pad, K), dtype=np.float32)
    w_pad[:N_FULL] = w
    ws_pad = np.ones((N_CORES * NBLK, KB), dtype=np.float32)
    ws_pad[:ws.shape[0]] = ws
    b_pad = np.zeros((n_pad,), dtype=np.float32)
    b_pad[:N_FULL] = b

    in_maps = []
    for c in range(N_CORES):
        wsr = np.ascontiguousarray(np.broadcast_to(
            ws_pad[c * NBLK:(c + 1) * NBLK].reshape(1, NBLK * KB),
            (128, NBLK * KB)))
        biasr = np.ascontiguousarray(np.broadcast_to(
            b_pad[c * N_SH:(c + 1) * N_SH][None, :], (128, N_SH)))
        in_maps.append({
            "x": x2d,
            "w": np.ascontiguousarray(w_pad[c * N_SH:(c + 1) * N_SH]),
            "wsr": wsr,
            "biasr": biasr,
        })
    return in_maps


def kernel(x, weight, weight_scale, bias):
    from concourse.bass_utils import run_bass_kernel_spmd

    nc = _get_nc()
    in_maps = _make_in_maps(x, weight, weight_scale, bias)
    res = run_bass_kernel_spmd(nc, in_maps, list(range(N_CORES)))
    y = np.concatenate([res.results[c]["y"] for c in range(N_CORES)], axis=1)
    out_shape = tuple(np.asarray(x).shape[:-1]) + (N_FULL,)
    return np.ascontiguousarray(y[:, :N_FULL]).reshape(out_shape)


# revision 23
# speedup vs baseline: 52.2747x; 52.2747x over previous
"""FP8 block-quantized linear (nn_FP8Linear) on 8 trn2 NeuronCores.

y = dequant(fp8_blockquant(x)) @ dequant(fp8(weight) * block_scales).T + bias

Sharding: column-parallel (tensor-parallel along out_features N).
N = 11008 = 86 blocks of 128 = 8 cores x 1376 rows (exact, no padding).
x is replicated; no collectives.

Device kernel (per core):
  - weight: load fp32 -> cast fp8e4 (RNE; identical to OCP e4m3fn for
    |w| <= 240, incl. subnormals - both bias-7) -> multiply per-(128x128)
    block scales (per-partition rows pre-gathered on host) -> bf16 ->
    SBUF xbar-transpose into K-major resident tiles (full chunks are 4D
    (128, nt, KB, 128) so each transpose has a contiguous destination;
    matmuls read a strided 3D moving AP).
  - x per 128-row M-tile: load fp32 -> per-(row,128-block) absmax (DVE) ->
    scales 224/max (half the reference's 448/max so values fit TRN fp8e4's
    +-240 range; the fp8 grid is relative so quantization is identical) ->
    quantize to fp8e4 (DVE) -> dequantize xq*(max/224) -> bf16 on the ACT
    engine (32 per-k-block Copy activations with per-partition scale) ->
    DMA-xbar-transpose to (128K, 32kb, 128M) -> per n-chunk: bias-init
    matmul (K=1, float32r) + 32 accumulating bf16 matmuls -> ACT copies
    PSUM->SBUF -> store fp32.
"""

import numpy as np

# ---- problem constants (hardcoded per contract) ----
M2D = 8192        # 4*2048 flattened tokens
K = 4096
N_FULL = 11008
N_CORES = 8
N_SH = N_FULL // N_CORES   # 1376 out-features per core
KB = K // 128              # 32 k-blocks
FP8_HW_MAX = 224.0  # reference uses 448 (e4m3fn); TRN fp8e4 tops at 240

_CACHE = {}


def _chunks(n_sh):
    # chunks of <=512 made of whole-or-partial 128-tiles; a trailing partial
    # tile becomes its own chunk so full chunks stay 128-aligned
    out = []
    off = 0
    while off < n_sh:
        w = min(512, n_sh - off)
        if w % 128 and w > 128:
            w -= w % 128
        out.append((off, w))
        off += w
    return out


def _build_nc(m2d=M2D, n_sh=N_SH, reps=1):
    import concourse.mybir as mybir
    import concourse.tile as tile
    from concourse import bacc

    f32 = mybir.dt.float32
    f32r = mybir.dt.float32r
    bf16 = mybir.dt.bfloat16
    f8 = mybir.dt.float8e4
    Copy = mybir.ActivationFunctionType.Copy

    chunks = _chunks(n_sh)
    n_tiles = (n_sh + 127) // 128
    n_mt = m2d // 128

    nc = bacc.Bacc("TRN2", target_bir_lowering=False, debug=False,
                   num_devices=N_CORES)

    x_d = nc.dram_tensor("x", (m2d, K), f32, kind="ExternalInput")
    w_d = nc.dram_tensor("w", (n_sh, K), f32, kind="ExternalInput")
    wsr_d = nc.dram_tensor("wsr", (n_tiles * 128, KB), f32,
                           kind="ExternalInput")
    bias_d = nc.dram_tensor("bias", (1, n_sh), f32, kind="ExternalInput")
    y_d = nc.dram_tensor("y", (m2d, n_sh), f32, kind="ExternalOutput")

    x_ap = x_d.ap()
    w_ap = w_d.ap()
    y_ap = y_d.ap()

    # map each 128-row N-tile to its chunk and offset within the chunk
    def tile_chunk(nb):
        for j, (off, wd) in enumerate(chunks):
            if off <= nb * 128 < off + wd:
                return j, nb * 128 - off
        raise AssertionError

    with tile.TileContext(nc) as tc:
        with (
            tc.tile_pool(name="const", bufs=1) as constp,
            tc.tile_pool(name="wT", bufs=1) as wTp,
            tc.tile_pool(name="ld", bufs=3) as ldp,
            tc.tile_pool(name="q8", bufs=2) as q8p,
            tc.tile_pool(name="deq", bufs=3) as deqp,
            tc.tile_pool(name="xT", bufs=2) as xTp,
            tc.tile_pool(name="sc", bufs=2) as scp,
            tc.tile_pool(name="yo", bufs=2) as yop,
            tc.tile_pool(name="ps", bufs=8, space="PSUM") as psp,
        ):
            # constants
            ws_sb = constp.tile([128, n_tiles, KB], f32)
            nc.sync.dma_start(
                out=ws_sb[:],
                in_=wsr_d.ap().rearrange("(t p) k -> p t k", p=128))
            bias_sb = constp.tile([1, n_sh], f32)
            nc.sync.dma_start(out=bias_sb[:], in_=bias_d.ap())
            ones_sb = constp.tile([1, 128], f32)
            nc.vector.memset(ones_sb[:], 1.0)
            # f32r copies (walrus requires matmul f32r inputs to come from a
            # rounding producer, not a bitcast view)
            bias_r = constp.tile([1, n_sh], f32r)
            nc.vector.tensor_copy(out=bias_r[:], in_=bias_sb[:])
            ones_r = constp.tile([1, 128], f32r)
            nc.vector.tensor_copy(out=ones_r[:], in_=ones_sb[:])

            # ---- weight pipeline (all on-chip) ----
            # load fp32 -> fp8 cast (ACT) -> *block-scale -> bf16 (DVE) ->
            # SBUF xbar-transpose straight into the chunk's K-major tile.
            # Full chunks are 4D (128, nt, KB, 128) so every transpose has a
            # contiguous destination; matmul reads a strided 3D moving AP.
            wT = []
            for j, (off, wd) in enumerate(chunks):
                if wd % 128 == 0:
                    t = wTp.tile([128, wd // 128, KB, 128], bf16,
                                 tag=f"wT{j}", name=f"wT{j}")
                else:
                    t = wTp.tile([128, KB, wd], bf16, tag=f"wT{j}",
                                 name=f"wT{j}")
                wT.append(t)
            # software-pipelined emission: the ACT queue is in-order, so the
            # transpose of tile nb-1 is emitted after the cast of tile nb to
            # keep per-nb stages overlapped instead of serialized.
            pending_transpose = None
            for nb in range(n_tiles):
                pv = min(128, n_sh - nb * 128)
                wld = ldp.tile([128, K], f32, tag="ld", name=f"wld{nb}")
                nc.sync.dma_start(out=wld[:pv], in_=w_ap[nb * 128:nb * 128 + pv, :])
                wq = q8p.tile([128, K], f8, tag="q8", name=f"wq{nb}")
                nc.scalar.activation(out=wq[:pv], in_=wld[:pv], func=Copy)
                if pending_transpose is not None:
                    pending_transpose()
                wdq = deqp.tile([128, K], bf16, tag="deq", name=f"wdq{nb}")
                nc.vector.tensor_tensor(
                    out=wdq[:pv].rearrange("p (a b) -> p a b", b=128),
                    in0=wq[:pv].rearrange("p (a b) -> p a b", b=128),
                    in1=ws_sb[:pv, nb, :, None].broadcast_to([pv, KB, 128]),
                    op=mybir.AluOpType.mult,
                )

                def make_transpose(nb=nb, pv=pv, wdq=wdq):
                    def emit():
                        j, loc = tile_chunk(nb)
                        if chunks[j][1] % 128 == 0:
                            nc.scalar.dma_start(out=wT[j][:, loc // 128],
                                                in_=wdq[:pv], transpose=True)
                        else:
                            nc.scalar.dma_start(out=wT[j][:], in_=wdq[:pv],
                                                transpose=True)
                    return emit
                pending_transpose = make_transpose()
            pending_transpose()

            def rhs_ap(j, kb):
                if chunks[j][1] % 128 == 0:
                    return wT[j][:, :, kb, :]
                return wT[j][:, kb, :]

            # ---- main M loop ----
            def emit_m_loop():
              for mt in range(n_mt):
                xld = ldp.tile([128, K], f32, tag="ld", name=f"xld{mt}")
                nc.sync.dma_start(out=xld[:], in_=x_ap[mt * 128:(mt + 1) * 128, :])

                mx = scp.tile([128, KB], f32, tag="mx", name=f"mx{mt}")
                nc.vector.tensor_reduce(
                    out=mx[:],
                    in_=xld.rearrange("p (a b) -> p a b", b=128),
                    axis=mybir.AxisListType.X,
                    op=mybir.AluOpType.max,
                    apply_absolute_value=True,
                )
                # guard all-zero blocks (reference would NaN; inputs are randn)
                nc.vector.tensor_scalar(out=mx[:], in0=mx[:], scalar1=1e-30,
                                        scalar2=None, op0=mybir.AluOpType.max)
                r2 = scp.tile([128, KB], f32, tag="r2", name=f"r2{mt}")
                nc.vector.reciprocal(r2[:], mx[:])
                nc.vector.tensor_scalar(out=r2[:], in0=r2[:], scalar1=FP8_HW_MAX,
                                        scalar2=None, op0=mybir.AluOpType.mult)
                s2 = scp.tile([128, KB], f32, tag="s2", name=f"s2{mt}")
                nc.vector.tensor_scalar(out=s2[:], in0=mx[:],
                                        scalar1=1.0 / FP8_HW_MAX,
                                        scalar2=None, op0=mybir.AluOpType.mult)

                xq = q8p.tile([128, K], f8, tag="q8", name=f"xq{mt}")
                nc.vector.tensor_tensor(
                    out=xq.rearrange("p (a b) -> p a b", b=128),
                    in0=xld.rearrange("p (a b) -> p a b", b=128),
                    in1=r2[:, :, None].broadcast_to([128, KB, 128]),
                    op=mybir.AluOpType.mult,
                )
                # dequantize on ACT: per k-block Copy with per-partition scale
                xdq = deqp.tile([128, K], bf16, tag="deq", name=f"xdq{mt}")
                for kb in range(KB):
                    nc.scalar.activation(
                        out=xdq[:, kb * 128:(kb + 1) * 128],
                        in_=xq[:, kb * 128:(kb + 1) * 128],
                        func=Copy,
                        scale=s2[:, kb:kb + 1],
                    )
                xT = xTp.tile([128, KB, 128], bf16, tag="xT", name=f"xT{mt}")
                nc.scalar.dma_start(out=xT[:], in_=xdq[:], transpose=True)

                yo = yop.tile([128, n_sh], f32, tag="yo", name=f"yo{mt}")
                pss = [psp.tile([128, 512], f32, tag="ps", name=f"ps_{mt}_{j}")
                       for j in range(len(chunks))]
                # bias init via K=1 float32r matmul (broadcasts bias row)
                for j, (off, wd) in enumerate(chunks):
                    nc.tensor.matmul(
                        pss[j][:, :wd],
                        ones_r[:],
                        bias_r[:, off:off + wd],
                        start=True, stop=False,
                    )
                if mt < 4:
                    # during weight preload, consume chunks in readiness order
                    for j, (off, wd) in enumerate(chunks):
                        for kb in range(KB):
                            nc.tensor.matmul(
                                pss[j][:, :wd], xT[:, kb, :], rhs_ap(j, kb),
                                start=False, stop=(kb == KB - 1))
                else:
                    for kb in range(KB):
                        for j, (off, wd) in enumerate(chunks):
                            nc.tensor.matmul(
                                pss[j][:, :wd], xT[:, kb, :], rhs_ap(j, kb),
                                start=False, stop=(kb == KB - 1))
                for j, (off, wd) in enumerate(chunks):
                    nc.scalar.activation(out=yo[:, off:off + wd],
                                         in_=pss[j][:, :wd], func=Copy)
                nc.sync.dma_start(out=y_ap[mt * 128:(mt + 1) * 128, :], in_=yo[:])

            if reps == 1:
                emit_m_loop()
            else:
                with tc.For_i(0, reps, 1):
                    emit_m_loop()

    nc.compile()
    return nc


def _get_nc(m2d=M2D, n_sh=N_SH, reps=1):
    key = (m2d, n_sh, reps)
    if key not in _CACHE:
        _CACHE[key] = _build_nc(m2d, n_sh, reps)
    return _CACHE[key]


def _make_in_maps(x, weight, weight_scale, bias):
    x2d = np.ascontiguousarray(
        np.asarray(x, dtype=np.float32).reshape(M2D, K))
    w = np.asarray(weight, dtype=np.float32)
    ws = np.asarray(weight_scale, dtype=np.float32)
    b = np.asarray(bias, dtype=np.float32)

    n_tiles = (N_SH + 127) // 128
    in_maps = []
    for c in range(N_CORES):
        # per-row scale gather: row r of this shard uses global block row
        # (c*N_SH + r) // 128; rows beyond the shard get a harmless 1.0
        r = np.arange(n_tiles * 128)
        gblk = (c * N_SH + np.minimum(r, N_SH - 1)) // 128
        wsr = ws[gblk].astype(np.float32)
        wsr[r >= N_SH] = 1.0
        in_maps.append({
            "x": x2d,
            "w": np.ascontiguousarray(w[c * N_SH:(c + 1) * N_SH]),
            "wsr": np.ascontiguousarray(wsr),
            "bias": np.ascontiguousarray(
                b[c * N_SH:(c + 1) * N_SH][None, :]),
        })
    return in_maps


def kernel(x, weight, weight_scale, bias):
    from concourse.bass_utils import run_bass_kernel_spmd

    nc = _get_nc()
    in_maps = _make_in_maps(x, weight, weight_scale, bias)
    res = run_bass_kernel_spmd(nc, in_maps, list(range(N_CORES)))
    y = np.concatenate([res.results[c]["y"] for c in range(N_CORES)], axis=1)
    out_shape = tuple(np.asarray(x).shape[:-1]) + (N_FULL,)
    return np.ascontiguousarray(y).reshape(out_shape)
